# revision 13
# baseline (speedup 1.0000x reference)
"""GraphSAGE/GraphConv (DGL norm='both') Bass kernel for 8 Trainium2 cores.

Math (reference):
  x[n,f]   : node features, n in [0,160000), f in [0,64)   (from inputs[8,64,20000])
  agg[d]   = norm_dst[d] * sum_{e: dst[e]=d} norm_src[src[e]] * x[src[e]]
  out      = leaky_relu(agg @ W + b, 0.01), returned as [8,64,20000] feature-major.

Device strategy (per core, vertex-cut on dst):
  - core c owns dst nodes [c*20000,(c+1)*20000) == output slice c of dim 0.
  - x table in DRAM pre-scaled by norm_src (per-node degree normalization);
    norm_dst applied on device after aggregation (column scale on Z).
  - dst tiles of 128 nodes, grouped in bands of BAND tiles; edges sorted by
    (band, class of 32768 src rows, tile, src).
  - gather phase per (band,class): large dma_gather batches (GSUB chunks,
    ~3K descriptors) on 2 SWDGE queues with an enlarged descriptor carveout;
    gathered rows buffered in SBUF (bf16 via bulk ACT cast) for the band.
  - matmul phase tile-major: each tile's chunk matmuls are contiguous, so
    PSUM accumulation groups never interleave. 128-edge chunks may STRADDLE
    a tile boundary (second matmul into t+1); per-edge labels dla/dlb
    (255 sentinel = no match) route edges, so padding is only per-(b,c,t)
    inter-core max.
  - DVE builds one-hots (tensor_tensor is_equal broadcast) and applies
    norm_dst + leaky; ACT casts/evacuates and adds bias.
"""

import os

import numpy as np
import ml_dtypes

from concourse import bass, mybir
import concourse.bacc as bacc
from concourse.tile import TileContext
from concourse.bass_utils import run_bass_kernel_spmd

BF16 = ml_dtypes.bfloat16
F32 = np.float32

LAST_RESULTS = None  # test harness introspection (exec time / trace)

CHUNK = 128     # edges per matmul chunk (PE contraction dim)
TW = 128        # dst-tile width (one-hot columns / PSUM free dim)
BAND = 12       # dst tiles per band (gather/compute overlap granularity)
WCLS = 32768    # src index window (int16 range for dma_gather)
GSUB = 7        # chunks per dma_gather instruction (896 descriptors, ring cap 1023)
SENT = 255.0    # dla/dlb sentinel: never matches iota 0..127
DMA_SCRATCH = 16384   # SWDGE descriptor carveout (default 1023-desc ring)
NQUEUES = 1


def _build_layout(src, dst, n_nodes, n_cores, npc):
    """Static shared chunk grid + per-core slot arrays."""
    nt = -(-npc // TW)                       # 157
    nb = -(-nt // BAND)
    ncls = -(-n_nodes // WCLS)               # 5

    owner = dst // npc
    rem = dst - owner * npc
    tile = rem // TW
    dl = (rem - tile * TW).astype(np.float32)
    band = tile // BAND
    cls = src // WCLS
    srcl = (src - cls * WCLS).astype(np.int16)

    key = ((owner * nb + band) * ncls + cls) * nt + tile
    nseg = n_cores * nb * ncls * nt
    counts = np.bincount(key, minlength=nseg).reshape(n_cores, nb, ncls, nt)
    T = counts.max(axis=0)                   # [nb, ncls, nt] static seg sizes

    band_tiles = [list(range(b * BAND, min((b + 1) * BAND, nt))) for b in range(nb)]
    seg_off = np.zeros((nb, ncls, nt), np.int64)
    L = np.zeros((nb, ncls), np.int64)
    for b in range(nb):
        ts = band_tiles[b]
        for c in range(ncls):
            run = 0
            for t in ts:
                seg_off[b, c, t] = run
                run += int(T[b, c, t])
            L[b, c] = run
            for t in ts[:-1]:
                assert T[b, c, t] >= CHUNK or T[b, c, t] == 0, (b, c, t, T[b, c, t])

    G = -(-L // CHUNK)                       # chunks per (band, cls)
    g0 = np.zeros((nb, ncls), np.int64)
    acc = 0
    for b in range(nb):
        for c in range(ncls):
            g0[b, c] = acc
            acc += int(G[b, c])
    M = int(acc)

    tA = np.zeros(M, np.int64)
    straddle = np.zeros(M, bool)
    grp_c = np.zeros(M, np.int64)
    j_of_m = np.zeros(M, np.int64)
    for b in range(nb):
        ts = band_tiles[b]
        for c in range(ncls):
            if G[b, c] == 0:
                continue
            bounds = np.asarray(
                [seg_off[b, c, t] for t in ts] + [L[b, c]], np.int64
            )
            for j in range(int(G[b, c])):
                m = int(g0[b, c] + j)
                grp_c[m] = c
                j_of_m[m] = j
                p0 = j * CHUNK
                p1 = min(p0 + CHUNK - 1, int(L[b, c]) - 1)
                if p0 >= L[b, c]:
                    tA[m] = ts[-1]
                    continue
                ia = min(int(np.searchsorted(bounds, p0, side="right")) - 1,
                         len(ts) - 1)
                ib = min(int(np.searchsorted(bounds, p1, side="right")) - 1,
                         len(ts) - 1)
                tA[m] = ts[ia]
                straddle[m] = ib > ia
                assert ib - ia <= 1, (b, c, j, ia, ib)

    # per-core slot fill (vectorized)
    order = np.lexsort((src, key))
    key_s = key[order]
    seg_start = np.zeros(nseg + 1, np.int64)
    np.cumsum(counts.reshape(-1), out=seg_start[1:])
    rank = np.arange(len(order), dtype=np.int64) - seg_start[key_s]
    b_s = (key_s // (ncls * nt)) % nb
    c_s = key_s // (nb * ncls * nt)
    cls_s = (key_s // nt) % ncls
    t_s = key_s % nt
    pos = seg_off[b_s, cls_s, t_s] + rank
    m_s = g0[b_s, cls_s] + pos // CHUNK
    p_s = pos % CHUNK

    tile_sorted = tile[order]
    dl_sorted = dl[order]
    ta_of_m = tA[m_s]
    is_a = tile_sorted == ta_of_m
    is_b = tile_sorted == ta_of_m + 1
    assert np.all(is_a | is_b), "edge outside its chunk's 2-tile window"

    idx_slot = np.zeros((n_cores, CHUNK, M), np.int16)
    dla = np.full((n_cores, CHUNK, M), SENT, BF16)
    dlb = np.full((n_cores, CHUNK, M), SENT, BF16)
    idx_slot[c_s, p_s, m_s] = srcl[order]
    dla[c_s[is_a], p_s[is_a], m_s[is_a]] = dl_sorted[is_a].astype(BF16)
    dlb[c_s[is_b], p_s[is_b], m_s[is_b]] = dl_sorted[is_b].astype(BF16)

    # dma_gather index stream: flat chunk-major (m*128+p), wrapped into 16
    # partitions and replicated x8. Column range for chunk m is m*8..m*8+8.
    idx16 = np.zeros((n_cores, 128, M * (CHUNK // 16)), np.int16)
    for cc in range(n_cores):
        flat_i = idx_slot[cc].T.reshape(-1)
        wrapped = flat_i.reshape(-1, 16).T
        idx16[cc] = np.tile(wrapped, (8, 1))

    # emission bookkeeping: per tile, ordered list of (m, which)
    emits = [[] for _ in range(nt)]
    for b in range(nb):
        for c in range(ncls):
            for j in range(int(G[b, c])):
                m = int(g0[b, c] + j)
                emits[int(tA[m])].append((m, 0))
                if straddle[m]:
                    emits[int(tA[m]) + 1].append((m, 1))
    for t in range(nt):
        assert emits[t], f"tile {t} has no chunks"

    plan = dict(
        nt=nt, nb=nb, ncls=ncls, M=M, band_tiles=band_tiles,
        g0=g0, G=G, grp_c=grp_c, j_of_m=j_of_m, emits=emits,
    )
    return idx16, dla, dlb, plan


def _build_nc(n_nodes, feat, outd, npc, plan, n_cores):
    f32 = mybir.dt.float32
    bf16 = mybir.dt.bfloat16
    i16 = mybir.dt.int16

    nt, nb, ncls, M = plan["nt"], plan["nb"], plan["ncls"], plan["M"]
    band_tiles = plan["band_tiles"]
    g0, G = plan["g0"], plan["G"]
    grp_c, j_of_m, emits = plan["grp_c"], plan["j_of_m"], plan["emits"]

    nc = bacc.Bacc(
        "TRN2",
        target_bir_lowering=False,
        debug=False,
        enable_asserts=False,
        num_devices=n_cores,
        dynamic_dma_scratch_size=DMA_SCRATCH,
        num_swdge_queues=NQUEUES,
    )

    x_t = nc.dram_tensor("x_tab", [n_nodes, feat], f32, kind="ExternalInput")
    idx_t = nc.dram_tensor("idx16", [128, M * (CHUNK // 16)], i16, kind="ExternalInput")
    dla_t = nc.dram_tensor("dla", [128, M], bf16, kind="ExternalInput")
    dlb_t = nc.dram_tensor("dlb", [128, M], bf16, kind="ExternalInput")
    iota_t = nc.dram_tensor("iota", [128, TW], bf16, kind="ExternalInput")
    nd_t = nc.dram_tensor("ndst", [outd, npc], bf16, kind="ExternalInput")
    W_t = nc.dram_tensor("Wt", [feat, outd], bf16, kind="ExternalInput")
    b_t = nc.dram_tensor("bias", [outd, 1], f32, kind="ExternalInput")
    out_t = nc.dram_tensor("out", [outd, npc], f32, kind="ExternalOutput")

    eq = mybir.AluOpType.is_equal
    mult = mybir.AluOpType.mult

    qn = 0
    with TileContext(nc) as tc:
        with (
            tc.tile_pool(name="const", bufs=1) as constp,
            tc.tile_pool(name="gidx", bufs=2) as gidxp,
            tc.tile_pool(name="gbuf", bufs=2) as gpool,
            tc.tile_pool(name="gb16", bufs=2 * ncls) as gb16p,
            tc.tile_pool(name="onehot", bufs=8) as spool,
            tc.tile_pool(name="evac", bufs=4) as evacp,
            tc.tile_pool(name="ndst", bufs=2) as ndstp,
            tc.tile_pool(name="stage", bufs=2) as stagep,
            tc.tile_pool(name="stage2", bufs=2) as stage2p,
            tc.tile_pool(name="leak", bufs=2) as lkp,
            tc.tile_pool(name="psA", bufs=4, space="PSUM") as psA,
            tc.tile_pool(name="psZ", bufs=2, space="PSUM") as psZ,
        ):
            dla_sb = constp.tile([128, M], bf16)
            nc.sync.dma_start(dla_sb[:], dla_t[:])
            dlb_sb = constp.tile([128, M], bf16)
            nc.sync.dma_start(dlb_sb[:], dlb_t[:])
            iota_sb = constp.tile([128, TW], bf16)
            nc.sync.dma_start(iota_sb[:], iota_t[:])
            W_sb = constp.tile([feat, outd], bf16)
            nc.sync.dma_start(W_sb[:], W_t[:])
            b_sb = constp.tile([outd, 1], f32)
            nc.sync.dma_start(b_sb[:], b_t[:])

            for b in range(nb):
                ts = band_tiles[b]
                ntb = len(ts)
                col0 = ts[0] * TW
                col1 = min(npc, (ts[-1] + 1) * TW)
                ndb = ndstp.tile([outd, ntb * TW], bf16, tag="nd")
                nc.sync.dma_start(ndb[:, : col1 - col0], nd_t[:, col0:col1])
                stage = stagep.tile([outd, ntb * TW], f32, tag="st")

                gbufs = {}
                for c in range(ncls):
                    K = int(G[b, c])
                    if K == 0:
                        continue
                    a = int(g0[b, c])
                    row0 = c * WCLS
                    row1 = min(n_nodes, (c + 1) * WCLS)
                    it = gidxp.tile([128, K * (CHUNK // 16)], i16, tag="gi")
                    nc.sync.dma_start(
                        it[:],
                        idx_t[:, a * (CHUNK // 16):(a + K) * (CHUNK // 16)],
                    )
                    gb = gb16p.tile([128, K * feat], bf16, tag="gc",
                                    name=f"gb{b}_{c}")
                    for s0 in range(0, K, GSUB):
                        ks = min(GSUB, K - s0)
                        gt = gpool.tile([128, GSUB * feat], f32, tag="g")
                        nc.gpsimd.dma_gather(
                            out_ap=gt[:, :ks * feat].rearrange(
                                "p (k f) -> p k f", f=feat),
                            in_ap=x_t[row0:row1, :],
                            idxs_ap=it[:, s0 * (CHUNK // 16):(s0 + ks) * (CHUNK // 16)],
                            num_idxs=ks * CHUNK,
                            num_idxs_reg=ks * CHUNK,
                            elem_size=feat,
                            queue_num=qn,
                        )
                        qn = (qn + 1) % NQUEUES
                        nc.scalar.activation(
                            gb[:, s0 * feat:(s0 + ks) * feat],
                            gt[:, :ks * feat],
                            mybir.ActivationFunctionType.Copy,
                        )
                    gbufs[c] = gb

                for lt, t in enumerate(ts):
                    agg = psA.tile([feat, TW], f32, tag="agg", name=f"agg{t}")
                    n_em = len(emits[t])
                    for i, (m, w) in enumerate(emits[t]):
                        dlx_sb = dla_sb if w == 0 else dlb_sb
                        c = int(grp_c[m])
                        j = int(j_of_m[m])
                        st = spool.tile([128, TW], bf16, tag="s")
                        nc.vector.tensor_tensor(
                            out=st[:],
                            in0=iota_sb[:],
                            in1=dlx_sb[:, m:m + 1].to_broadcast([128, TW]),
                            op=eq,
                        )
                        nc.tensor.matmul(
                            out=agg[:],
                            lhsT=gbufs[c][:, j * feat:(j + 1) * feat],
                            rhs=st[:],
                            start=(i == 0),
                            stop=(i == n_em - 1),
                        )
                    ev = evacp.tile([feat, TW], bf16, tag="ev")
                    nc.scalar.activation(
                        ev[:], agg[:], mybir.ActivationFunctionType.Copy
                    )
                    zt = psZ.tile([outd, TW], f32, tag="z")
                    nc.tensor.matmul(
                        out=zt[:], lhsT=W_sb[:], rhs=ev[:], start=True, stop=True
                    )
                    nc.scalar.activation(
                        stage[:, lt * TW:(lt + 1) * TW],
                        zt[:],
                        mybir.ActivationFunctionType.Copy,
                    )

                # norm_dst scale, bias, leaky-relu (band-wide)
                nc.vector.tensor_tensor(
                    out=stage[:], in0=stage[:], in1=ndb[:], op=mult,
                )
                stage2 = stage2p.tile([outd, ntb * TW], f32, tag="s2")
                nc.scalar.activation(
                    stage2[:],
                    stage[:],
                    mybir.ActivationFunctionType.Identity,
                    bias=b_sb[:],
                )
                zs = lkp.tile([outd, ntb * TW], f32, tag="zs")
                nc.vector.tensor_scalar(
                    out=zs[:], in0=stage2[:], scalar1=0.01, scalar2=None, op0=mult,
                )
                nc.vector.tensor_tensor(
                    out=stage2[:], in0=stage2[:], in1=zs[:],
                    op=mybir.AluOpType.max,
                )
                nc.sync.dma_start(
                    out_t[:, col0:col1], stage2[:, : col1 - col0]
                )

    nc.compile()
    return nc


def _prep(inputs, W, b, src, dst, n_cores):
    sli, feat, node = inputs.shape
    n_nodes = sli * node
    outd = W.shape[1]
    npc = n_nodes // n_cores

    src = np.asarray(src).astype(np.int64)
    dst = np.asarray(dst).astype(np.int64)
    deg_out = np.bincount(src, minlength=n_nodes)
    deg_in = np.bincount(dst, minlength=n_nodes)
    norm_src = np.maximum(deg_out, 1).astype(F32) ** -0.5
    norm_dst = np.maximum(deg_in, 1).astype(F32) ** -0.5

    # node-major table pre-scaled by norm_src (degree normalization)
    x_tab = np.ascontiguousarray(
        np.asarray(inputs, dtype=F32).transpose(0, 2, 1).reshape(n_nodes, feat)
        * norm_src[:, None]
    )

    idx16, dla, dlb, plan = _build_layout(src, dst, n_nodes, n_cores, npc)

    iota = np.broadcast_to(np.arange(TW, dtype=F32), (128, TW)).astype(BF16)
    Wt = np.asarray(W, dtype=F32).astype(BF16)
    bias = np.asarray(b, dtype=F32).reshape(outd, 1)

    in_maps = []
    for c in range(n_cores):
        nd_rep = np.broadcast_to(
            norm_dst[c * npc:(c + 1) * npc][None, :], (outd, npc)
        ).astype(BF16)
        in_maps.append(
            {
                "x_tab": x_tab,
                "idx16": np.ascontiguousarray(idx16[c]),
                "dla": np.ascontiguousarray(dla[c]),
                "dlb": np.ascontiguousarray(dlb[c]),
                "iota": np.ascontiguousarray(iota),
                "ndst": np.ascontiguousarray(nd_rep),
                "Wt": Wt,
                "bias": bias,
            }
        )
    meta = dict(n_nodes=n_nodes, feat=feat, outd=outd, npc=npc,
                sli=sli, node=node, plan=plan)
    return in_maps, meta


def kernel(inputs, W, b, src, dst):
    global LAST_RESULTS
    n_cores = 8
    inputs = np.asarray(inputs, dtype=F32)
    in_maps, meta = _prep(inputs, W, b, src, dst, n_cores)

    nc = _build_nc(
        meta["n_nodes"], meta["feat"], meta["outd"], meta["npc"],
        meta["plan"], n_cores,
    )

    res = run_bass_kernel_spmd(
        nc,
        in_maps,
        core_ids=list(range(n_cores)),
        trace=bool(int(os.environ.get("KERNEL_TRACE", "0"))),
    )
    LAST_RESULTS = res

    out = np.stack([r["out"] for r in res.results], axis=0)  # [8, 64, 20000]
    return out.astype(F32)


# revision 15
# speedup vs baseline: 1.1609x; 1.1609x over previous
"""GraphSAGE/GraphConv (DGL norm='both') Bass kernel for 8 Trainium2 cores.

Math (reference):
  x[n,f]   : node features, n in [0,160000), f in [0,64)   (from inputs[8,64,20000])
  agg[d]   = norm_dst[d] * sum_{e: dst[e]=d} norm_src[src[e]] * x[src[e]]
  out      = leaky_relu(agg @ W + b, 0.01), returned as [8,64,20000] feature-major.

Device strategy (per core, vertex-cut on dst):
  - core c owns dst nodes [c*20000,(c+1)*20000) == output slice c of dim 0.
  - x table in DRAM pre-scaled by norm_src (per-node degree normalization);
    norm_dst applied on device after aggregation (column scale on Z).
  - dst tiles of 128 nodes, grouped in bands of BAND tiles; edges sorted by
    (band, class of 32768 src rows, tile, src).
  - gather phase per (band,class): large dma_gather batches (GSUB chunks,
    ~3K descriptors) on 2 SWDGE queues with an enlarged descriptor carveout;
    gathered rows buffered in SBUF (bf16 via bulk ACT cast) for the band.
  - matmul phase tile-major: each tile's chunk matmuls are contiguous, so
    PSUM accumulation groups never interleave. 128-edge chunks may STRADDLE
    a tile boundary (second matmul into t+1); per-edge labels dla/dlb
    (255 sentinel = no match) route edges, so padding is only per-(b,c,t)
    inter-core max.
  - DVE builds one-hots (tensor_tensor is_equal broadcast) and applies
    norm_dst + leaky; ACT casts/evacuates and adds bias.
"""

import os

import numpy as np
import ml_dtypes

from concourse import bass, mybir
import concourse.bacc as bacc
from concourse.tile import TileContext
from concourse.bass_utils import run_bass_kernel_spmd

BF16 = ml_dtypes.bfloat16
F32 = np.float32

LAST_RESULTS = None  # test harness introspection (exec time / trace)

CHUNK = 128     # edges per matmul chunk (PE contraction dim)
TW = 128        # dst-tile width (one-hot columns / PSUM free dim)
BAND = 12       # dst tiles per band (gather/compute overlap granularity)
WCLS = 32768    # src index window (int16 range for dma_gather)
GSUB = 4        # chunks per dma_gather instruction (512 descriptors)
SENT = 255.0    # dla/dlb sentinel: never matches iota 0..127
DMA_SCRATCH = 16384   # SWDGE descriptor carveout (default 1023-desc ring)
NQUEUES = 1


def _build_layout(src, dst, n_nodes, n_cores, npc):
    """Static shared chunk grid + per-core slot arrays."""
    nt = -(-npc // TW)                       # 157
    nb = -(-nt // BAND)
    ncls = -(-n_nodes // WCLS)               # 5

    owner = dst // npc
    rem = dst - owner * npc
    tile = rem // TW
    dl = (rem - tile * TW).astype(np.float32)
    band = tile // BAND
    cls = src // WCLS
    srcl = (src - cls * WCLS).astype(np.int16)

    key = ((owner * nb + band) * ncls + cls) * nt + tile
    nseg = n_cores * nb * ncls * nt
    counts = np.bincount(key, minlength=nseg).reshape(n_cores, nb, ncls, nt)
    T = counts.max(axis=0)                   # [nb, ncls, nt] static seg sizes

    band_tiles = [list(range(b * BAND, min((b + 1) * BAND, nt))) for b in range(nb)]
    seg_off = np.zeros((nb, ncls, nt), np.int64)
    L = np.zeros((nb, ncls), np.int64)
    for b in range(nb):
        ts = band_tiles[b]
        for c in range(ncls):
            run = 0
            for t in ts:
                seg_off[b, c, t] = run
                run += int(T[b, c, t])
            L[b, c] = run
            for t in ts[:-1]:
                assert T[b, c, t] >= CHUNK or T[b, c, t] == 0, (b, c, t, T[b, c, t])

    G = -(-L // CHUNK)                       # chunks per (band, cls)
    g0 = np.zeros((nb, ncls), np.int64)
    acc = 0
    for b in range(nb):
        for c in range(ncls):
            g0[b, c] = acc
            acc += int(G[b, c])
    M = int(acc)

    tA = np.zeros(M, np.int64)
    straddle = np.zeros(M, bool)
    grp_c = np.zeros(M, np.int64)
    j_of_m = np.zeros(M, np.int64)
    for b in range(nb):
        ts = band_tiles[b]
        for c in range(ncls):
            if G[b, c] == 0:
                continue
            bounds = np.asarray(
                [seg_off[b, c, t] for t in ts] + [L[b, c]], np.int64
            )
            for j in range(int(G[b, c])):
                m = int(g0[b, c] + j)
                grp_c[m] = c
                j_of_m[m] = j
                p0 = j * CHUNK
                p1 = min(p0 + CHUNK - 1, int(L[b, c]) - 1)
                if p0 >= L[b, c]:
                    tA[m] = ts[-1]
                    continue
                ia = min(int(np.searchsorted(bounds, p0, side="right")) - 1,
                         len(ts) - 1)
                ib = min(int(np.searchsorted(bounds, p1, side="right")) - 1,
                         len(ts) - 1)
                tA[m] = ts[ia]
                straddle[m] = ib > ia
                assert ib - ia <= 1, (b, c, j, ia, ib)

    # per-core slot fill (vectorized)
    order = np.lexsort((src, key))
    key_s = key[order]
    seg_start = np.zeros(nseg + 1, np.int64)
    np.cumsum(counts.reshape(-1), out=seg_start[1:])
    rank = np.arange(len(order), dtype=np.int64) - seg_start[key_s]
    b_s = (key_s // (ncls * nt)) % nb
    c_s = key_s // (nb * ncls * nt)
    cls_s = (key_s // nt) % ncls
    t_s = key_s % nt
    pos = seg_off[b_s, cls_s, t_s] + rank
    m_s = g0[b_s, cls_s] + pos // CHUNK
    p_s = pos % CHUNK

    tile_sorted = tile[order]
    dl_sorted = dl[order]
    ta_of_m = tA[m_s]
    is_a = tile_sorted == ta_of_m
    is_b = tile_sorted == ta_of_m + 1
    assert np.all(is_a | is_b), "edge outside its chunk's 2-tile window"

    idx_slot = np.zeros((n_cores, CHUNK, M), np.int16)
    dla = np.full((n_cores, CHUNK, M), SENT, BF16)
    dlb = np.full((n_cores, CHUNK, M), SENT, BF16)
    idx_slot[c_s, p_s, m_s] = srcl[order]
    dla[c_s[is_a], p_s[is_a], m_s[is_a]] = dl_sorted[is_a].astype(BF16)
    dlb[c_s[is_b], p_s[is_b], m_s[is_b]] = dl_sorted[is_b].astype(BF16)

    # dma_gather index stream: flat chunk-major (m*128+p), wrapped into 16
    # partitions and replicated x8. Column range for chunk m is m*8..m*8+8.
    idx16 = np.zeros((n_cores, 128, M * (CHUNK // 16)), np.int16)
    for cc in range(n_cores):
        flat_i = idx_slot[cc].T.reshape(-1)
        wrapped = flat_i.reshape(-1, 16).T
        idx16[cc] = np.tile(wrapped, (8, 1))

    # emission bookkeeping: per tile, ordered list of (m, which)
    emits = [[] for _ in range(nt)]
    for b in range(nb):
        for c in range(ncls):
            for j in range(int(G[b, c])):
                m = int(g0[b, c] + j)
                emits[int(tA[m])].append((m, 0))
                if straddle[m]:
                    emits[int(tA[m]) + 1].append((m, 1))
    for t in range(nt):
        assert emits[t], f"tile {t} has no chunks"

    plan = dict(
        nt=nt, nb=nb, ncls=ncls, M=M, band_tiles=band_tiles,
        g0=g0, G=G, grp_c=grp_c, j_of_m=j_of_m, emits=emits,
    )
    return idx16, dla, dlb, plan


def _build_nc(n_nodes, feat, outd, npc, plan, n_cores):
    f32 = mybir.dt.float32
    bf16 = mybir.dt.bfloat16
    i16 = mybir.dt.int16

    nt, nb, ncls, M = plan["nt"], plan["nb"], plan["ncls"], plan["M"]
    band_tiles = plan["band_tiles"]
    g0, G = plan["g0"], plan["G"]
    grp_c, j_of_m, emits = plan["grp_c"], plan["j_of_m"], plan["emits"]

    nc = bacc.Bacc(
        "TRN2",
        target_bir_lowering=False,
        debug=False,
        enable_asserts=False,
        num_devices=n_cores,
        dynamic_dma_scratch_size=DMA_SCRATCH,
        num_swdge_queues=NQUEUES,
    )

    x_t = nc.dram_tensor("x_tab", [n_nodes, feat], f32, kind="ExternalInput")
    idx_t = nc.dram_tensor("idx16", [128, M * (CHUNK // 16)], i16, kind="ExternalInput")
    dla_t = nc.dram_tensor("dla", [128, M], bf16, kind="ExternalInput")
    dlb_t = nc.dram_tensor("dlb", [128, M], bf16, kind="ExternalInput")
    iota_t = nc.dram_tensor("iota", [128, TW], bf16, kind="ExternalInput")
    nd_t = nc.dram_tensor("ndst", [outd, npc], bf16, kind="ExternalInput")
    W_t = nc.dram_tensor("Wt", [feat, outd], bf16, kind="ExternalInput")
    b_t = nc.dram_tensor("bias", [outd, 1], f32, kind="ExternalInput")
    out_t = nc.dram_tensor("out", [outd, npc], f32, kind="ExternalOutput")

    eq = mybir.AluOpType.is_equal
    mult = mybir.AluOpType.mult

    qn = 0
    with TileContext(nc) as tc:
        with (
            tc.tile_pool(name="const", bufs=1) as constp,
            tc.tile_pool(name="gidx", bufs=2) as gidxp,
            tc.tile_pool(name="gbuf", bufs=2) as gpool,
            tc.tile_pool(name="gb16", bufs=2 * ncls) as gb16p,
            tc.tile_pool(name="onehot", bufs=8) as spool,
            tc.tile_pool(name="evac", bufs=4) as evacp,
            tc.tile_pool(name="ndst", bufs=2) as ndstp,
            tc.tile_pool(name="stage", bufs=2) as stagep,
            tc.tile_pool(name="stage2", bufs=2) as stage2p,
            tc.tile_pool(name="leak", bufs=2) as lkp,
            tc.tile_pool(name="psA", bufs=4, space="PSUM") as psA,
            tc.tile_pool(name="psZ", bufs=2, space="PSUM") as psZ,
        ):
            dla_sb = constp.tile([128, M], bf16)
            nc.sync.dma_start(dla_sb[:], dla_t[:])
            dlb_sb = constp.tile([128, M], bf16)
            nc.sync.dma_start(dlb_sb[:], dlb_t[:])
            iota_sb = constp.tile([128, TW], bf16)
            nc.sync.dma_start(iota_sb[:], iota_t[:])
            W_sb = constp.tile([feat, outd], bf16)
            nc.sync.dma_start(W_sb[:], W_t[:])
            b_sb = constp.tile([outd, 1], f32)
            nc.sync.dma_start(b_sb[:], b_t[:])

            for b in range(nb):
                ts = band_tiles[b]
                ntb = len(ts)
                col0 = ts[0] * TW
                col1 = min(npc, (ts[-1] + 1) * TW)
                ndb = ndstp.tile([outd, ntb * TW], bf16, tag="nd")
                nc.sync.dma_start(ndb[:, : col1 - col0], nd_t[:, col0:col1])
                stage = stagep.tile([outd, ntb * TW], f32, tag="st")

                gbufs = {}
                for c in range(ncls):
                    K = int(G[b, c])
                    if K == 0:
                        continue
                    a = int(g0[b, c])
                    row0 = c * WCLS
                    row1 = min(n_nodes, (c + 1) * WCLS)
                    it = gidxp.tile([128, K * (CHUNK // 16)], i16, tag="gi")
                    nc.sync.dma_start(
                        it[:],
                        idx_t[:, a * (CHUNK // 16):(a + K) * (CHUNK // 16)],
                    )
                    gb = gb16p.tile([128, K * feat], bf16, tag="gc",
                                    name=f"gb{b}_{c}")
                    for s0 in range(0, K, GSUB):
                        ks = min(GSUB, K - s0)
                        gt = gpool.tile([128, GSUB * feat], f32, tag="g")
                        nc.gpsimd.dma_gather(
                            out_ap=gt[:, :ks * feat].rearrange(
                                "p (k f) -> p k f", f=feat),
                            in_ap=x_t[row0:row1, :],
                            idxs_ap=it[:, s0 * (CHUNK // 16):(s0 + ks) * (CHUNK // 16)],
                            num_idxs=ks * CHUNK,
                            num_idxs_reg=ks * CHUNK,
                            elem_size=feat,
                            queue_num=qn,
                        )
                        qn = (qn + 1) % NQUEUES
                        nc.scalar.activation(
                            gb[:, s0 * feat:(s0 + ks) * feat],
                            gt[:, :ks * feat],
                            mybir.ActivationFunctionType.Copy,
                        )
                    gbufs[c] = gb

                for lt, t in enumerate(ts):
                    agg = psA.tile([feat, TW], f32, tag="agg", name=f"agg{t}")
                    n_em = len(emits[t])
                    for i, (m, w) in enumerate(emits[t]):
                        dlx_sb = dla_sb if w == 0 else dlb_sb
                        c = int(grp_c[m])
                        j = int(j_of_m[m])
                        st = spool.tile([128, TW], bf16, tag="s")
                        nc.vector.tensor_tensor(
                            out=st[:],
                            in0=iota_sb[:],
                            in1=dlx_sb[:, m:m + 1].to_broadcast([128, TW]),
                            op=eq,
                        )
                        nc.tensor.matmul(
                            out=agg[:],
                            lhsT=gbufs[c][:, j * feat:(j + 1) * feat],
                            rhs=st[:],
                            start=(i == 0),
                            stop=(i == n_em - 1),
                        )
                    ev = evacp.tile([feat, TW], bf16, tag="ev")
                    nc.scalar.activation(
                        ev[:], agg[:], mybir.ActivationFunctionType.Copy
                    )
                    zt = psZ.tile([outd, TW], f32, tag="z")
                    nc.tensor.matmul(
                        out=zt[:], lhsT=W_sb[:], rhs=ev[:], start=True, stop=True
                    )
                    nc.scalar.activation(
                        stage[:, lt * TW:(lt + 1) * TW],
                        zt[:],
                        mybir.ActivationFunctionType.Copy,
                    )

                # norm_dst scale, bias, leaky-relu (band-wide)
                nc.vector.tensor_tensor(
                    out=stage[:], in0=stage[:], in1=ndb[:], op=mult,
                )
                stage2 = stage2p.tile([outd, ntb * TW], f32, tag="s2")
                nc.scalar.activation(
                    stage2[:],
                    stage[:],
                    mybir.ActivationFunctionType.Identity,
                    bias=b_sb[:],
                )
                zs = lkp.tile([outd, ntb * TW], f32, tag="zs")
                nc.vector.tensor_scalar(
                    out=zs[:], in0=stage2[:], scalar1=0.01, scalar2=None, op0=mult,
                )
                nc.vector.tensor_tensor(
                    out=stage2[:], in0=stage2[:], in1=zs[:],
                    op=mybir.AluOpType.max,
                )
                nc.sync.dma_start(
                    out_t[:, col0:col1], stage2[:, : col1 - col0]
                )

    nc.compile()
    return nc


def _prep(inputs, W, b, src, dst, n_cores):
    sli, feat, node = inputs.shape
    n_nodes = sli * node
    outd = W.shape[1]
    npc = n_nodes // n_cores

    src = np.asarray(src).astype(np.int64)
    dst = np.asarray(dst).astype(np.int64)
    deg_out = np.bincount(src, minlength=n_nodes)
    deg_in = np.bincount(dst, minlength=n_nodes)
    norm_src = np.maximum(deg_out, 1).astype(F32) ** -0.5
    norm_dst = np.maximum(deg_in, 1).astype(F32) ** -0.5

    # node-major table pre-scaled by norm_src (degree normalization)
    x_tab = np.ascontiguousarray(
        np.asarray(inputs, dtype=F32).transpose(0, 2, 1).reshape(n_nodes, feat)
        * norm_src[:, None]
    )

    idx16, dla, dlb, plan = _build_layout(src, dst, n_nodes, n_cores, npc)

    iota = np.broadcast_to(np.arange(TW, dtype=F32), (128, TW)).astype(BF16)
    Wt = np.asarray(W, dtype=F32).astype(BF16)
    bias = np.asarray(b, dtype=F32).reshape(outd, 1)

    in_maps = []
    for c in range(n_cores):
        nd_rep = np.broadcast_to(
            norm_dst[c * npc:(c + 1) * npc][None, :], (outd, npc)
        ).astype(BF16)
        in_maps.append(
            {
                "x_tab": x_tab,
                "idx16": np.ascontiguousarray(idx16[c]),
                "dla": np.ascontiguousarray(dla[c]),
                "dlb": np.ascontiguousarray(dlb[c]),
                "iota": np.ascontiguousarray(iota),
                "ndst": np.ascontiguousarray(nd_rep),
                "Wt": Wt,
                "bias": bias,
            }
        )
    meta = dict(n_nodes=n_nodes, feat=feat, outd=outd, npc=npc,
                sli=sli, node=node, plan=plan)
    return in_maps, meta


def kernel(inputs, W, b, src, dst):
    global LAST_RESULTS
    n_cores = 8
    inputs = np.asarray(inputs, dtype=F32)
    in_maps, meta = _prep(inputs, W, b, src, dst, n_cores)

    nc = _build_nc(
        meta["n_nodes"], meta["feat"], meta["outd"], meta["npc"],
        meta["plan"], n_cores,
    )

    res = run_bass_kernel_spmd(
        nc,
        in_maps,
        core_ids=list(range(n_cores)),
        trace=bool(int(os.environ.get("KERNEL_TRACE", "0"))),
    )
    LAST_RESULTS = res

    out = np.stack([r["out"] for r in res.results], axis=0)  # [8, 64, 20000]
    return out.astype(F32)
